# revision 20
# baseline (speedup 1.0000x reference)
"""Trainium2 Bass kernel for LittleBitLinear reconstruction (fp8 DoubleRow).

Computes M = (sign(U_fp) * ell) @ sign(V_fp)^T * g[None, :] * h[:, None]
for U_fp (4096, 1024), V_fp (11008, 1024) -> M (4096, 11008) fp32.

Strategy: shard d_in (rows of V_fp / columns of M) across 8 cores; U_fp, h,
ell replicated. Each core computes the full 4096 x 1376 column block.

The contraction-dim scale ell is split per-r into a product of two fp8-e4m3
grid values ell_r ~= a_r * b_r (pair-optimized on host over the e4m3 grid, a
few % the error of a single fp8 round). The matmul then runs entirely in
fp8 with DoubleRow perf mode (~1.44x bf16 matmul throughput):
  A[r, o] = sign(U^T)[r, o] * a_r          (fp8, exact: a_r on grid)
  B[r, i] = sign(V^T)[r, i] * b_r          (fp8, exact; sign(ell) folded in b)
  psum    = A-slice^T @ B                  (DoubleRow, fp32 PSUM)
  out     = bf16(psum * h_o) * bf16(g_i)   (ACT evac + DVE mult, bf16 store)
Host upcasts bf16 -> fp32. Measured end-to-end absmax rel err ~7.8e-3.

Sign+scale staging runs on DVE from the bf16 operands (exact arithmetic):
  s = (x >= 0) in {1, 0};  +-c = s * 2c - c  (one fused tensor_scalar).
Engine split: DVE staging + g-mult, ACT single merged PSUM evac per o-block
(one 3-bank [128,1376] read with per-partition h scale), GPSIMD issues the
output-store DMAs. A staging is chunked along d_out (512/512/1024/2048) and
interleaved into the o-block loop so matmuls start after ~1/8 of the U load.
"""

import os
import sys

import numpy as np

for _p in ("/opt/trn_rl_repo",):
    if _p not in sys.path and os.path.isdir(_p):
        sys.path.insert(0, _p)

D_OUT, D_IN, R, NCORES = 4096, 11008, 1024, 8
N_SH = D_IN // NCORES  # 1376
P = 128
KB = R // P            # 8 k-planes
NTILES = [(0, 512), (512, 512), (1024, 352)]
CHUNKS = [(0, 1024), (1024, 1024), (2048, 1024), (3072, 1024)]  # (col0, width)


def build_program():
    """Build the per-core Bass program (SPMD: same program, different data)."""
    from contextlib import ExitStack

    import concourse.bass as bass  # noqa: F401
    import concourse.mybir as mybir
    import concourse.tile as tile
    from concourse import bacc

    f32 = mybir.dt.float32
    bf16 = mybir.dt.bfloat16
    fp8 = mybir.dt.float8e4
    AF = mybir.ActivationFunctionType
    ALU = mybir.AluOpType
    DR = mybir.MatmulPerfMode.DoubleRow

    oblocks = D_OUT // P   # 32

    # o-block j -> (chunk q, index jq within chunk)
    ob_chunk = {}
    jj = 0
    for q, (c0, w) in enumerate(CHUNKS):
        for i in range(w // P):
            ob_chunk[jj] = (q, i)
            jj += 1
    assert jj == oblocks

    # staging schedule: chunk q's 8 planes spread over chunk q-1's o-blocks
    stage_at = {j: [] for j in range(oblocks)}
    for q in range(1, len(CHUNKS)):
        prev_obs = [j for j in range(oblocks) if ob_chunk[j][0] == q - 1]
        # spread over all but the last o-block so the final plane lands a
        # full o-block before the chunk's first matmul needs it
        span = prev_obs[:-1] if len(prev_obs) > 1 else prev_obs
        for k in range(KB):
            j = span[(k * len(span)) // KB]
            stage_at[j].append((q, k))

    nc = bacc.Bacc(None, target_bir_lowering=False)
    ut = nc.declare_dram_parameter("ut", [KB, P, D_OUT], bf16, isOutput=False)
    vt = nc.declare_dram_parameter("vt", [KB, P, N_SH], bf16, isOutput=False)
    hh = nc.declare_dram_parameter("h", [P, oblocks], f32, isOutput=False)
    gg = nc.declare_dram_parameter("g", [P, N_SH], bf16, isOutput=False)
    asc = nc.declare_dram_parameter("asc", [P, KB], f32, isOutput=False)
    asc2 = nc.declare_dram_parameter("asc2", [P, KB], f32, isOutput=False)
    bsc = nc.declare_dram_parameter("bsc", [P, KB], f32, isOutput=False)
    bsc2 = nc.declare_dram_parameter("bsc2", [P, KB], f32, isOutput=False)
    out = nc.declare_dram_parameter("out", [D_OUT, N_SH], bf16, isOutput=True)

    with tile.TileContext(nc) as tc, ExitStack() as ctx:
        consts = ctx.enter_context(tc.tile_pool(name="consts", bufs=1))
        vstg = ctx.enter_context(tc.tile_pool(name="vstg", bufs=2))
        vsgn = ctx.enter_context(tc.tile_pool(name="vsgn", bufs=4))
        bpool = ctx.enter_context(tc.tile_pool(name="bpool", bufs=1))
        ustg = ctx.enter_context(tc.tile_pool(name="ustg", bufs=2))
        usgn = ctx.enter_context(tc.tile_pool(name="usgn", bufs=4))
        apool = ctx.enter_context(tc.tile_pool(name="apool", bufs=2))
        outp = ctx.enter_context(tc.tile_pool(name="outp", bufs=3))
        outg = ctx.enter_context(tc.tile_pool(name="outg", bufs=3))
        psum = ctx.enter_context(tc.tile_pool(name="psum", bufs=2, space="PSUM"))

        # Route shared operands through one ACT copy so downstream DVE ops
        # carry a single cross-proc wait (walrus TT/TS struct holds only one
        # sync-wait slot).
        # consts ride the gpsimd DMA queue so vt/ut loads lead the sync queue
        def shared_const(name, param, shape, dt):
            raw = consts.tile(shape, dt, name=f"{name}_raw")
            nc.gpsimd.dma_start(out=raw, in_=param[:, :])
            sb = consts.tile(shape, dt, name=f"{name}_sb")
            nc.vector.tensor_scalar(
                out=sb, in0=raw, scalar1=0.0, scalar2=None, op0=ALU.add,
            )
            return sb

        b_sb = shared_const("b", bsc, [P, KB], f32)
        b2_sb = shared_const("b2", bsc2, [P, KB], f32)
        a_sb = shared_const("a", asc, [P, KB], f32)
        a2_sb = shared_const("a2", asc2, [P, KB], f32)

        # B = sign(V^T) * b_r in fp8, all 8 k-planes in one 3D tile.
        # Emitted interleaved with A chunk-0 staging below; even planes take
        # the ACT-Sign route, odd planes the DVE is_ge route, so the prologue
        # splits across both engines.
        bt = bpool.tile([P, KB, N_SH], fp8, name="bt")

        def stage_b_pair(d):
            # one DMA per dstep pair: halves sync-queue issue serialization,
            # and a pair is exactly what each DoubleRow step consumes
            vp = vstg.tile([P, 2, N_SH], bf16, tag="vstg", name=f"vp_{d}")
            nc.sync.dma_start(
                out=vp, in_=vt[2 * d:2 * d + 2, :, :].transpose([1, 0, 2])
            )
            for i in range(2):
                k = 2 * d + i
                vst = vp[:, i, :]
                vs = vsgn.tile([P, N_SH], bf16, tag="vsgn", name=f"vs_{k}")
                if k % 2 == 0:
                    nc.scalar.activation(out=vs, in_=vst, func=AF.Sign)
                    nc.vector.tensor_scalar(
                        out=bt[:, k, :], in0=vs, scalar1=b_sb[:, k:k + 1],
                        scalar2=None, op0=ALU.mult,
                    )
                else:
                    nc.vector.tensor_scalar(
                        out=vs, in0=vst, scalar1=0.0, scalar2=None,
                        op0=ALU.is_ge,
                    )
                    nc.vector.tensor_scalar(
                        out=bt[:, k, :], in0=vs,
                        scalar1=b2_sb[:, k:k + 1], scalar2=b_sb[:, k:k + 1],
                        op0=ALU.mult, op1=ALU.subtract,
                    )

        # A chunk staging, all on DVE: is_ge then fused affine (*2a - a) + fp8
        atiles = {}

        def stage_a_plane(q, k, act_route=False):
            c0, w = CHUNKS[q]
            if q not in atiles:
                atiles[q] = apool.tile(
                    [P, KB, w], fp8, tag=f"a{w}", name=f"a_{q}"
                )
            at = atiles[q]
            ust = ustg.tile([P, w], bf16, tag=f"ustg{w}", name=f"ust_{q}_{k}")
            nc.sync.dma_start(out=ust, in_=ut[k, :, c0:c0 + w])
            us = usgn.tile([P, w], bf16, tag=f"usgn{w}", name=f"us_{q}_{k}")
            if act_route:
                nc.scalar.activation(out=us, in_=ust, func=AF.Sign)
                nc.vector.tensor_scalar(
                    out=at[:, k, :], in0=us, scalar1=a_sb[:, k:k + 1],
                    scalar2=None, op0=ALU.mult,
                )
            else:
                nc.vector.tensor_scalar(
                    out=us, in0=ust, scalar1=0.0, scalar2=None, op0=ALU.is_ge,
                )
                nc.vector.tensor_scalar(
                    out=at[:, k, :], in0=us,
                    scalar1=a2_sb[:, k:k + 1], scalar2=a_sb[:, k:k + 1],
                    op0=ALU.mult, op1=ALU.subtract,
                )

        # prologue: pairwise-interleave B and A chunk-0 pairs so o-block 0's
        # first accumulation steps unblock as early as possible
        c0q0, wq0 = CHUNKS[0]
        atiles[0] = apool.tile([P, KB, wq0], fp8, tag=f"a{wq0}", name="a_0")

        def stage_a0_pair(d):
            up = ustg.tile([P, 2, wq0], bf16, tag=f"ustg{wq0}", name=f"up_{d}")
            nc.sync.dma_start(
                out=up, in_=ut[2 * d:2 * d + 2, :, c0q0:c0q0 + wq0].transpose([1, 0, 2])
            )
            for i in range(2):
                k = 2 * d + i
                ust = up[:, i, :]
                us = usgn.tile([P, wq0], bf16, tag=f"usgn{wq0}", name=f"us_0_{k}")
                if k % 2 == 1:
                    nc.scalar.activation(out=us, in_=ust, func=AF.Sign)
                    nc.vector.tensor_scalar(
                        out=atiles[0][:, k, :], in0=us, scalar1=a_sb[:, k:k + 1],
                        scalar2=None, op0=ALU.mult,
                    )
                else:
                    nc.vector.tensor_scalar(
                        out=us, in0=ust, scalar1=0.0, scalar2=None,
                        op0=ALU.is_ge,
                    )
                    nc.vector.tensor_scalar(
                        out=atiles[0][:, k, :], in0=us,
                        scalar1=a2_sb[:, k:k + 1], scalar2=a_sb[:, k:k + 1],
                        op0=ALU.mult, op1=ALU.subtract,
                    )

        for d in range(KB // 2):
            stage_b_pair(d)
            stage_a0_pair(d)

        # h/g are first needed by o-block 0's evac/g-mult; load them after the
        # staging ops are queued so they don't delay the prologue
        h_sb = shared_const("h", hh, [P, oblocks], f32)
        g_sb = shared_const("g", gg, [P, N_SH], bf16)

        # PE warm-up: dummy matmuls with no data deps run during the staging
        # prologue, so the HAM clock gate is already at 8/8 when the real
        # matmul stream begins (saves the ~3.4us cold-clock ramp).
        wl = consts.tile([P, 2, P], fp8, name="wl")
        nc.vector.memset(wl, 1.0)
        wr = consts.tile([P, 2, 512], fp8, name="wr")
        nc.vector.memset(wr, 1.0)
        pw = ctx.enter_context(tc.tile_pool(name="psumw", bufs=1, space="PSUM"))
        pwt = pw.tile([P, 512], f32, name="pwt")
        for _ in range(36):
            nc.tensor.matmul(
                pwt, lhsT=wl[:, :, :], rhs=wr[:, :, :],
                start=True, stop=True, perf_mode=DR,
            )

        def emit_mms(j, pt):
            q, jq = ob_chunk[j]
            at = atiles[q]
            for d in range(KB // 2):
                lhsT = at[:, 2 * d:2 * d + 2, jq * P:(jq + 1) * P]
                for n, (c0, nw) in enumerate(NTILES):
                    nc.tensor.matmul(
                        pt[:, c0:c0 + nw], lhsT=lhsT,
                        rhs=bt[:, 2 * d:2 * d + 2, c0:c0 + nw],
                        start=(d == 0), stop=(d == KB // 2 - 1),
                        perf_mode=DR,
                    )

        def emit_mms_pair(j0, j1, pt0, pt1):
            # dstep-interleaved across two o-blocks: during the staging ramp
            # each newly staged plane-pair unblocks twice the matmul work.
            # Dummy-MM bursts between dsteps keep the PE clock warm while the
            # next plane-pair is still staging.
            q0, jq0 = ob_chunk[j0]
            q1, jq1 = ob_chunk[j1]
            for d in range(KB // 2):
                for (q, jq, pt) in ((q0, jq0, pt0), (q1, jq1, pt1)):
                    lhsT = atiles[q][:, 2 * d:2 * d + 2, jq * P:(jq + 1) * P]
                    for n, (c0, nw) in enumerate(NTILES):
                        nc.tensor.matmul(
                            pt[:, c0:c0 + nw], lhsT=lhsT,
                            rhs=bt[:, 2 * d:2 * d + 2, c0:c0 + nw],
                            start=(d == 0), stop=(d == KB // 2 - 1),
                            perf_mode=DR,
                        )
                if d < KB // 2 - 1:
                    for _ in range(8):
                        nc.tensor.matmul(
                            pwt, lhsT=wl[:, :, :], rhs=wr[:, :, :],
                            start=True, stop=True, perf_mode=DR,
                        )

        def emit_evac(j, pt):
            ot = outp.tile([P, N_SH], bf16, tag="out", name=f"ot_{j}")
            og = outg.tile([P, N_SH], bf16, tag="og", name=f"og_{j}")
            nc.scalar.activation(
                out=ot, in_=pt, func=AF.Copy, scale=h_sb[:, j:j + 1],
            )
            nc.vector.tensor_tensor(out=og, in0=ot, in1=g_sb, op=ALU.mult)
            # stores round-robin over all three DMA-capable queues so no
            # single queue backs up and the final drains run in parallel
            eng = (nc.gpsimd, nc.sync, nc.scalar)[j % 3]
            eng.dma_start(out=out[j * P:(j + 1) * P, :], in_=og)

        def new_psum(j):
            return psum.tile(
                [P, N_SH], f32, tag="ps", name=f"ps_{j}",
                padded_shape=[P, 1536],
            )

        for j in range(oblocks):
            for (qn, kn) in stage_at[j]:
                stage_a_plane(qn, kn)
            if j in (0, 2):
                pt0, pt1 = new_psum(j), new_psum(j + 1)
                emit_mms_pair(j, j + 1, pt0, pt1)
                emit_evac(j, pt0)
                emit_evac(j + 1, pt1)
            elif j in (1, 3):
                continue
            else:
                pt = new_psum(j)
                emit_mms(j, pt)
                emit_evac(j, pt)

    nc.compile()
    return nc


_NC_CACHE = {}


def _get_nc():
    if "nc" not in _NC_CACHE:
        _NC_CACHE["nc"] = build_program()
    return _NC_CACHE["nc"]


def _pair_split_ell(ell):
    """Split each |ell_r| into a product a_r * b_r of e4m3 grid values.

    Returns (a, b_signed) as float32; a > 0, sign(ell) folded into b.
    """
    import ml_dtypes

    f8 = ml_dtypes.float8_e4m3
    grid = np.arange(256, dtype=np.uint8).view(f8).astype(np.float64)
    pos = np.unique(grid[np.isfinite(grid) & (grid > 0)])  # 119 values

    t = np.abs(ell).astype(np.float64)                     # (R,)
    q = t[:, None] / pos[None, :]                          # (R, 119)
    b = np.asarray(q, dtype=np.float64).astype(f8).astype(np.float64)
    bad = ~np.isfinite(b)
    prod = pos[None, :] * np.where(bad, 0.0, b)
    err = np.abs(prod - t[:, None])
    err[bad] = np.inf
    i = np.argmin(err, axis=1)
    a = pos[i]
    bsel = b[np.arange(len(t)), i]
    return (
        a.astype(np.float32),
        (bsel * np.where(ell >= 0, 1.0, -1.0)).astype(np.float32),
    )


def _make_in_maps(U_fp, V_fp, h, g, ell):
    U_fp = np.ascontiguousarray(np.asarray(U_fp, dtype=np.float32))
    V_fp = np.ascontiguousarray(np.asarray(V_fp, dtype=np.float32))
    h = np.asarray(h, dtype=np.float32).reshape(-1)
    g = np.asarray(g, dtype=np.float32).reshape(-1)
    ell = np.asarray(ell, dtype=np.float32).reshape(-1)

    import ml_dtypes

    bf = ml_dtypes.bfloat16

    a, b = _pair_split_ell(ell)
    a_t = np.ascontiguousarray(a.reshape(KB, P).T)           # (128, 8)
    a2_t = np.ascontiguousarray((2.0 * a).reshape(KB, P).T)
    b_t = np.ascontiguousarray(b.reshape(KB, P).T)
    b2_t = np.ascontiguousarray((2.0 * b).reshape(KB, P).T)
    h_t = np.ascontiguousarray(h.reshape(D_OUT // P, P).T)   # (128, 32)

    ut = np.ascontiguousarray(U_fp.T).astype(bf).reshape(KB, P, D_OUT)

    in_maps = []
    for c in range(NCORES):
        sl = slice(c * N_SH, (c + 1) * N_SH)
        in_maps.append({
            "ut": ut,
            "vt": np.ascontiguousarray(V_fp[sl, :].T).astype(bf).reshape(KB, P, N_SH),
            "h": h_t,
            "g": np.ascontiguousarray(
                np.broadcast_to(g[sl].astype(bf).reshape(1, N_SH), (P, N_SH))
            ),
            "asc": a_t,
            "asc2": a2_t,
            "bsc": b_t,
            "bsc2": b2_t,
        })
    return in_maps


def run(U_fp, V_fp, h, g, ell, trace=False):
    """Run on 8 NeuronCores; returns (M, BassKernelResults)."""
    from concourse.bass_utils import run_bass_kernel_spmd

    nc = _get_nc()
    in_maps = _make_in_maps(U_fp, V_fp, h, g, ell)
    res = run_bass_kernel_spmd(nc, in_maps, list(range(NCORES)), trace=trace)
    M = np.concatenate(
        [np.asarray(res.results[c]["out"]).astype(np.float32) for c in range(NCORES)],
        axis=1,
    )
    return M, res


def kernel(U_fp, V_fp, h, g, ell):
    M, _ = run(U_fp, V_fp, h, g, ell, trace=False)
    return M


# revision 21
# speedup vs baseline: 1.0827x; 1.0827x over previous
"""Trainium2 Bass kernel for LittleBitLinear reconstruction (fp8 DoubleRow).

Computes M = (sign(U_fp) * ell) @ sign(V_fp)^T * g[None, :] * h[:, None]
for U_fp (4096, 1024), V_fp (11008, 1024) -> M (4096, 11008) fp32.

Strategy: shard d_in (rows of V_fp / columns of M) across 8 cores; U_fp, h,
ell replicated. Each core computes the full 4096 x 1376 column block.

The contraction-dim scale ell is split per-r into a product of two fp8-e4m3
grid values ell_r ~= a_r * b_r (pair-optimized on host over the e4m3 grid, a
few % the error of a single fp8 round). The matmul then runs entirely in
fp8 with DoubleRow perf mode (~1.44x bf16 matmul throughput):
  A[r, o] = sign(U^T)[r, o] * a_r          (fp8, exact: a_r on grid)
  B[r, i] = sign(V^T)[r, i] * b_r          (fp8, exact; sign(ell) folded in b)
  psum    = A-slice^T @ B                  (DoubleRow, fp32 PSUM)
  out     = bf16(psum * h_o) * bf16(g_i)   (ACT evac + DVE mult, bf16 store)
Host upcasts bf16 -> fp32. Measured end-to-end absmax rel err ~7.8e-3.

Sign+scale staging runs on DVE from the bf16 operands (exact arithmetic):
  s = (x >= 0) in {1, 0};  +-c = s * 2c - c  (one fused tensor_scalar).
Engine split: DVE staging + g-mult, ACT single merged PSUM evac per o-block
(one 3-bank [128,1376] read with per-partition h scale), GPSIMD issues the
output-store DMAs. A staging is chunked along d_out (512/512/1024/2048) and
interleaved into the o-block loop so matmuls start after ~1/8 of the U load.
"""

import os
import sys

import numpy as np

for _p in ("/opt/trn_rl_repo",):
    if _p not in sys.path and os.path.isdir(_p):
        sys.path.insert(0, _p)

D_OUT, D_IN, R, NCORES = 4096, 11008, 1024, 8
N_SH = D_IN // NCORES  # 1376
P = 128
KB = R // P            # 8 k-planes
NTILES = [(0, 512), (512, 512), (1024, 352)]
CHUNKS = [(0, 1024), (1024, 1024), (2048, 1024), (3072, 1024)]  # (col0, width)


def build_program():
    """Build the per-core Bass program (SPMD: same program, different data)."""
    from contextlib import ExitStack

    import concourse.bass as bass  # noqa: F401
    import concourse.mybir as mybir
    import concourse.tile as tile
    from concourse import bacc

    f32 = mybir.dt.float32
    bf16 = mybir.dt.bfloat16
    fp8 = mybir.dt.float8e4
    AF = mybir.ActivationFunctionType
    ALU = mybir.AluOpType
    DR = mybir.MatmulPerfMode.DoubleRow

    oblocks = D_OUT // P   # 32

    # o-block j -> (chunk q, index jq within chunk)
    ob_chunk = {}
    jj = 0
    for q, (c0, w) in enumerate(CHUNKS):
        for i in range(w // P):
            ob_chunk[jj] = (q, i)
            jj += 1
    assert jj == oblocks

    # staging schedule: chunk q's 8 planes spread over chunk q-1's o-blocks
    stage_at = {j: [] for j in range(oblocks)}
    for q in range(1, len(CHUNKS)):
        prev_obs = [j for j in range(oblocks) if ob_chunk[j][0] == q - 1]
        # spread over all but the last o-block so the final plane lands a
        # full o-block before the chunk's first matmul needs it
        span = prev_obs[:-1] if len(prev_obs) > 1 else prev_obs
        for k in range(KB):
            j = span[(k * len(span)) // KB]
            stage_at[j].append((q, k))

    nc = bacc.Bacc(None, target_bir_lowering=False)
    ut = nc.declare_dram_parameter("ut", [R, D_OUT], bf16, isOutput=False)
    vt = nc.declare_dram_parameter("vt", [R, N_SH], bf16, isOutput=False)
    hh = nc.declare_dram_parameter("h", [P, oblocks], f32, isOutput=False)
    gg = nc.declare_dram_parameter("g", [P, N_SH], bf16, isOutput=False)
    asc = nc.declare_dram_parameter("asc", [P, KB], f32, isOutput=False)
    asc2 = nc.declare_dram_parameter("asc2", [P, KB], f32, isOutput=False)
    bsc = nc.declare_dram_parameter("bsc", [P, KB], f32, isOutput=False)
    bsc2 = nc.declare_dram_parameter("bsc2", [P, KB], f32, isOutput=False)
    out = nc.declare_dram_parameter("out", [D_OUT, N_SH], bf16, isOutput=True)

    with tile.TileContext(nc) as tc, ExitStack() as ctx:
        consts = ctx.enter_context(tc.tile_pool(name="consts", bufs=1))
        vstg = ctx.enter_context(tc.tile_pool(name="vstg", bufs=4))
        vsgn = ctx.enter_context(tc.tile_pool(name="vsgn", bufs=4))
        bpool = ctx.enter_context(tc.tile_pool(name="bpool", bufs=1))
        ustg = ctx.enter_context(tc.tile_pool(name="ustg", bufs=4))
        usgn = ctx.enter_context(tc.tile_pool(name="usgn", bufs=4))
        apool = ctx.enter_context(tc.tile_pool(name="apool", bufs=2))
        outp = ctx.enter_context(tc.tile_pool(name="outp", bufs=3))
        outg = ctx.enter_context(tc.tile_pool(name="outg", bufs=3))
        psum = ctx.enter_context(tc.tile_pool(name="psum", bufs=2, space="PSUM"))

        # Route shared operands through one ACT copy so downstream DVE ops
        # carry a single cross-proc wait (walrus TT/TS struct holds only one
        # sync-wait slot).
        # consts ride the gpsimd DMA queue so vt/ut loads lead the sync queue
        def shared_const(name, param, shape, dt):
            raw = consts.tile(shape, dt, name=f"{name}_raw")
            nc.gpsimd.dma_start(out=raw, in_=param[:, :])
            sb = consts.tile(shape, dt, name=f"{name}_sb")
            nc.vector.tensor_scalar(
                out=sb, in0=raw, scalar1=0.0, scalar2=None, op0=ALU.add,
            )
            return sb

        b_sb = shared_const("b", bsc, [P, KB], f32)
        b2_sb = shared_const("b2", bsc2, [P, KB], f32)
        a_sb = shared_const("a", asc, [P, KB], f32)
        a2_sb = shared_const("a2", asc2, [P, KB], f32)

        # B = sign(V^T) * b_r in fp8, all 8 k-planes in one 3D tile.
        # Emitted interleaved with A chunk-0 staging below; even planes take
        # the ACT-Sign route, odd planes the DVE is_ge route, so the prologue
        # splits across both engines.
        bt = bpool.tile([P, KB, N_SH], fp8, name="bt")

        def stage_b_plane(k):
            vst = vstg.tile([P, N_SH], bf16, tag="vstg", name=f"vst_{k}")
            nc.sync.dma_start(out=vst, in_=vt[k * P:(k + 1) * P, :])
            vs = vsgn.tile([P, N_SH], bf16, tag="vsgn", name=f"vs_{k}")
            if k % 2 == 0:
                nc.scalar.activation(out=vs, in_=vst, func=AF.Sign)
                nc.vector.tensor_scalar(
                    out=bt[:, k, :], in0=vs, scalar1=b_sb[:, k:k + 1],
                    scalar2=None, op0=ALU.mult,
                )
            else:
                nc.vector.tensor_scalar(
                    out=vs, in0=vst, scalar1=0.0, scalar2=None, op0=ALU.is_ge,
                )
                nc.vector.tensor_scalar(
                    out=bt[:, k, :], in0=vs,
                    scalar1=b2_sb[:, k:k + 1], scalar2=b_sb[:, k:k + 1],
                    op0=ALU.mult, op1=ALU.subtract,
                )

        # A chunk staging, all on DVE: is_ge then fused affine (*2a - a) + fp8
        atiles = {}

        def stage_a_plane(q, k, act_route=False):
            c0, w = CHUNKS[q]
            if q not in atiles:
                atiles[q] = apool.tile(
                    [P, KB, w], fp8, tag=f"a{w}", name=f"a_{q}"
                )
            at = atiles[q]
            ust = ustg.tile([P, w], bf16, tag=f"ustg{w}", name=f"ust_{q}_{k}")
            nc.sync.dma_start(out=ust, in_=ut[k * P:(k + 1) * P, c0:c0 + w])
            us = usgn.tile([P, w], bf16, tag=f"usgn{w}", name=f"us_{q}_{k}")
            if act_route:
                nc.scalar.activation(out=us, in_=ust, func=AF.Sign)
                nc.vector.tensor_scalar(
                    out=at[:, k, :], in0=us, scalar1=a_sb[:, k:k + 1],
                    scalar2=None, op0=ALU.mult,
                )
            else:
                nc.vector.tensor_scalar(
                    out=us, in0=ust, scalar1=0.0, scalar2=None, op0=ALU.is_ge,
                )
                nc.vector.tensor_scalar(
                    out=at[:, k, :], in0=us,
                    scalar1=a2_sb[:, k:k + 1], scalar2=a_sb[:, k:k + 1],
                    op0=ALU.mult, op1=ALU.subtract,
                )

        # prologue: pairwise-interleave B and A chunk-0 planes so o-block 0's
        # first accumulation steps unblock as early as possible
        for k in range(KB):
            stage_b_plane(k)
            stage_a_plane(0, k, act_route=(k % 2 == 1))

        # h/g are first needed by o-block 0's evac/g-mult; load them after the
        # staging ops are queued so they don't delay the prologue
        h_sb = shared_const("h", hh, [P, oblocks], f32)
        g_sb = shared_const("g", gg, [P, N_SH], bf16)

        # PE warm-up: dummy matmuls with no data deps run during the staging
        # prologue, so the HAM clock gate is already at 8/8 when the real
        # matmul stream begins (saves the ~3.4us cold-clock ramp).
        wl = consts.tile([P, 2, P], fp8, name="wl")
        nc.vector.memset(wl, 1.0)
        wr = consts.tile([P, 2, 512], fp8, name="wr")
        nc.vector.memset(wr, 1.0)
        pw = ctx.enter_context(tc.tile_pool(name="psumw", bufs=1, space="PSUM"))
        pwt = pw.tile([P, 512], f32, name="pwt")
        for _ in range(36):
            nc.tensor.matmul(
                pwt, lhsT=wl[:, :, :], rhs=wr[:, :, :],
                start=True, stop=True, perf_mode=DR,
            )

        def emit_mms(j, pt):
            q, jq = ob_chunk[j]
            at = atiles[q]
            for d in range(KB // 2):
                lhsT = at[:, 2 * d:2 * d + 2, jq * P:(jq + 1) * P]
                for n, (c0, nw) in enumerate(NTILES):
                    nc.tensor.matmul(
                        pt[:, c0:c0 + nw], lhsT=lhsT,
                        rhs=bt[:, 2 * d:2 * d + 2, c0:c0 + nw],
                        start=(d == 0), stop=(d == KB // 2 - 1),
                        perf_mode=DR,
                    )

        def emit_mms_pair(j0, j1, pt0, pt1):
            # dstep-interleaved across two o-blocks: during the staging ramp
            # each newly staged plane-pair unblocks twice the matmul work.
            # Dummy-MM bursts between dsteps keep the PE clock warm while the
            # next plane-pair is still staging.
            q0, jq0 = ob_chunk[j0]
            q1, jq1 = ob_chunk[j1]
            for d in range(KB // 2):
                for (q, jq, pt) in ((q0, jq0, pt0), (q1, jq1, pt1)):
                    lhsT = atiles[q][:, 2 * d:2 * d + 2, jq * P:(jq + 1) * P]
                    for n, (c0, nw) in enumerate(NTILES):
                        nc.tensor.matmul(
                            pt[:, c0:c0 + nw], lhsT=lhsT,
                            rhs=bt[:, 2 * d:2 * d + 2, c0:c0 + nw],
                            start=(d == 0), stop=(d == KB // 2 - 1),
                            perf_mode=DR,
                        )
                if d < KB // 2 - 1:
                    for _ in range(8):
                        nc.tensor.matmul(
                            pwt, lhsT=wl[:, :, :], rhs=wr[:, :, :],
                            start=True, stop=True, perf_mode=DR,
                        )

        def emit_evac(j, pt):
            ot = outp.tile([P, N_SH], bf16, tag="out", name=f"ot_{j}")
            og = outg.tile([P, N_SH], bf16, tag="og", name=f"og_{j}")
            nc.scalar.activation(
                out=ot, in_=pt, func=AF.Copy, scale=h_sb[:, j:j + 1],
            )
            nc.vector.tensor_tensor(out=og, in0=ot, in1=g_sb, op=ALU.mult)
            # stores round-robin over all three DMA-capable queues so no
            # single queue backs up and the final drains run in parallel
            eng = (nc.gpsimd, nc.sync, nc.scalar)[j % 3]
            eng.dma_start(out=out[j * P:(j + 1) * P, :], in_=og)

        def new_psum(j):
            return psum.tile(
                [P, N_SH], f32, tag="ps", name=f"ps_{j}",
                padded_shape=[P, 1536],
            )

        for j in range(oblocks):
            for (qn, kn) in stage_at[j]:
                stage_a_plane(qn, kn)
            if j in (0, 2):
                pt0, pt1 = new_psum(j), new_psum(j + 1)
                emit_mms_pair(j, j + 1, pt0, pt1)
                emit_evac(j, pt0)
                emit_evac(j + 1, pt1)
            elif j in (1, 3):
                continue
            else:
                pt = new_psum(j)
                emit_mms(j, pt)
                emit_evac(j, pt)

    nc.compile()
    return nc


_NC_CACHE = {}


def _get_nc():
    if "nc" not in _NC_CACHE:
        _NC_CACHE["nc"] = build_program()
    return _NC_CACHE["nc"]


def _pair_split_ell(ell):
    """Split each |ell_r| into a product a_r * b_r of e4m3 grid values.

    Returns (a, b_signed) as float32; a > 0, sign(ell) folded into b.
    """
    import ml_dtypes

    f8 = ml_dtypes.float8_e4m3
    grid = np.arange(256, dtype=np.uint8).view(f8).astype(np.float64)
    pos = np.unique(grid[np.isfinite(grid) & (grid > 0)])  # 119 values

    t = np.abs(ell).astype(np.float64)                     # (R,)
    q = t[:, None] / pos[None, :]                          # (R, 119)
    b = np.asarray(q, dtype=np.float64).astype(f8).astype(np.float64)
    bad = ~np.isfinite(b)
    prod = pos[None, :] * np.where(bad, 0.0, b)
    err = np.abs(prod - t[:, None])
    err[bad] = np.inf
    i = np.argmin(err, axis=1)
    a = pos[i]
    bsel = b[np.arange(len(t)), i]
    return (
        a.astype(np.float32),
        (bsel * np.where(ell >= 0, 1.0, -1.0)).astype(np.float32),
    )


def _make_in_maps(U_fp, V_fp, h, g, ell):
    U_fp = np.ascontiguousarray(np.asarray(U_fp, dtype=np.float32))
    V_fp = np.ascontiguousarray(np.asarray(V_fp, dtype=np.float32))
    h = np.asarray(h, dtype=np.float32).reshape(-1)
    g = np.asarray(g, dtype=np.float32).reshape(-1)
    ell = np.asarray(ell, dtype=np.float32).reshape(-1)

    import ml_dtypes

    bf = ml_dtypes.bfloat16

    a, b = _pair_split_ell(ell)
    a_t = np.ascontiguousarray(a.reshape(KB, P).T)           # (128, 8)
    a2_t = np.ascontiguousarray((2.0 * a).reshape(KB, P).T)
    b_t = np.ascontiguousarray(b.reshape(KB, P).T)
    b2_t = np.ascontiguousarray((2.0 * b).reshape(KB, P).T)
    h_t = np.ascontiguousarray(h.reshape(D_OUT // P, P).T)   # (128, 32)

    ut = np.ascontiguousarray(U_fp.T).astype(bf)             # (R, D_OUT)

    in_maps = []
    for c in range(NCORES):
        sl = slice(c * N_SH, (c + 1) * N_SH)
        in_maps.append({
            "ut": ut,
            "vt": np.ascontiguousarray(V_fp[sl, :].T).astype(bf),  # (R, N_SH)
            "h": h_t,
            "g": np.ascontiguousarray(
                np.broadcast_to(g[sl].astype(bf).reshape(1, N_SH), (P, N_SH))
            ),
            "asc": a_t,
            "asc2": a2_t,
            "bsc": b_t,
            "bsc2": b2_t,
        })
    return in_maps


def run(U_fp, V_fp, h, g, ell, trace=False):
    """Run on 8 NeuronCores; returns (M, BassKernelResults)."""
    from concourse.bass_utils import run_bass_kernel_spmd

    nc = _get_nc()
    in_maps = _make_in_maps(U_fp, V_fp, h, g, ell)
    res = run_bass_kernel_spmd(nc, in_maps, list(range(NCORES)), trace=trace)
    M = np.concatenate(
        [np.asarray(res.results[c]["out"]).astype(np.float32) for c in range(NCORES)],
        axis=1,
    )
    return M, res


def kernel(U_fp, V_fp, h, g, ell):
    M, _ = run(U_fp, V_fp, h, g, ell, trace=False)
    return M
